# revision 17
# baseline (speedup 1.0000x reference)
"""Trainium2 Bass kernel for nn_Bilinear_54065048322517.

Math:  out[b, j] = input2[b, j] * sum_{i,k} weights[i, j, k] * input1[b, i]
           =   input2 * (input1 @ weights.sum(axis=2))
Shapes: input1 (16384, 64) f32, input2 (16384, 2048) f32,
        weights (64, 2048, 64) f32, out (16384, 2048) f32.

Sharding: split J=2048 into 8 shards of 256 (one per NeuronCore).
Each core reads: input1 full (4MB) + its input2 shard (16MB) + its
weights shard (4MB), writes its out shard (16MB) -> 40MB of HBM
traffic per core (vs 64.5MB for batch sharding, which would have to
replicate the 32MB weights).

Toolchain constraint: at most ONE embedded semaphore wait per
instruction.  Techniques used to honor it:
  - engine-proc instructions inherit semaphore observations from
    earlier same-engine instructions, so tiny REAL ops (1x1x1 PE
    matmuls into a psum scratch, 1-elem DVE copies) act as "joiners"
    that each absorb one foreign semaphore;
  - HWDGE DMAs inherit waits carried by earlier DMAs on the same
    ring (FIFO issue), so tiny 4-byte "flag" DMAs pre-carry lane
    semaphores, and loads/stores are ordered on the SP ring in
    consumption order; weight chunks ride the ACT ring;
  - add_dep_helper(sync=False) pins scheduler ordering.

Per-core kernel:
  phase A:
    - weights shard (64,256,64) loaded as 8 chunks (64, 2048) on the
      ACT ring; DVE grouped-reduce over K -> w2 (64, 256)
    - input1 loaded as (128, 8192) on the SP ring: partition p holds
      rows {256n + 2p + q} (512B contiguous runs); 128 TensorE
      transposes of (128, 64) blocks -> x1T0/x1T1 (64, 8192):
      x1Tq[i, n*128 + p] = x1[256n + 2p + q, i].
  phase B (groups of 4 super-tiles = 1024 rows):
    - DMA x2 group -> xtile (128, 2048), SP ring (2KB runs)
    - per super-tile n: 2 matmuls (K=64, M=128, N=256):
        pt[:, q*256:(q+1)*256] = x1Tq[:, n*128:+128].T @ w2
      (psum partition p of q-half <-> row 256n + 2p + q)
    - DVE: otile = pt * xtile
    - DMA otile -> out group, SP ring.
"""

import numpy as np

B, I, J, K = 16384, 64, 2048, 64
NCORES = 8
JS = J // NCORES          # 256 columns per core
NSUP = B // 256           # 64 super-tiles of 256 rows
GROUP = 4                 # super-tiles per DMA group (1MB per stream DMA)
NG = NSUP // GROUP        # 16 groups
NWCHUNK = 8               # weights load chunks
WBUFS = 4                 # resident weight chunk slots
XBUFS = 3                 # xtile buffer depth
OBUFS = 2                 # otile buffer depth

_CACHE = {}


def _build_nc():
    from contextlib import ExitStack

    import concourse.mybir as mybir
    import concourse.tile as tile
    from concourse import bacc, masks
    from concourse.tile import add_dep_helper

    f32 = mybir.dt.float32
    nc = bacc.Bacc()

    x1 = nc.dram_tensor("input1", [B, I], f32, kind="ExternalInput")
    x2 = nc.dram_tensor("input2", [B, JS], f32, kind="ExternalInput")
    w = nc.dram_tensor("weights", [I, JS, K], f32, kind="ExternalInput")
    out = nc.dram_tensor("out", [B, JS], f32, kind="ExternalOutput")
    scratch = nc.dram_tensor("flagscratch", [64], f32)  # flag-DMA sink

    with tile.TileContext(nc) as tc, ExitStack() as ctx:
        const_pool = ctx.enter_context(tc.tile_pool(name="const", bufs=1))
        stage_pool = ctx.enter_context(tc.tile_pool(name="stage", bufs=1))
        wc_pool = ctx.enter_context(tc.tile_pool(name="wc", bufs=WBUFS))
        x_pool = ctx.enter_context(tc.tile_pool(name="xin", bufs=XBUFS))
        o_pool = ctx.enter_context(tc.tile_pool(name="oout", bufs=OBUFS))
        ps_pool = ctx.enter_context(tc.tile_pool(name="ps", bufs=4, space="PSUM"))
        tr_pool = ctx.enter_context(tc.tile_pool(name="tr", bufs=1, space="PSUM"))
        sc_pool = ctx.enter_context(tc.tile_pool(name="sc", bufs=1, space="PSUM"))

        identity = const_pool.tile([128, 128], f32)
        masks.make_identity(nc, identity[:])
        pescratch = sc_pool.tile([128, 8], f32)
        dvescratch = const_pool.tile([128, 2 * NG + 2], f32)

        nflag = [0]
        last_ring = {"sp": None, "act": None}

        def ring_dma(ring, out_ap, in_ap):
            """DMA with pinned per-ring ordering."""
            eng = nc.sync if ring == "sp" else nc.scalar
            bi = eng.dma_start(out=out_ap, in_=in_ap)
            if last_ring[ring] is not None:
                add_dep_helper(bi.ins, last_ring[ring], False, "ring order")
            last_ring[ring] = bi.ins
            return bi.ins

        def flag_dma(ring, src_ap):
            """4-byte DMA that pre-carries a lane semaphore on the ring."""
            idx = nflag[0]
            nflag[0] += 1
            return ring_dma(ring, scratch[idx : idx + 1], src_ap)

        def pe_join(*aps):
            """Tiny real matmul on PE absorbing one foreign semaphore."""
            a = aps[0]
            b = aps[1] if len(aps) > 1 else aps[0]
            bi = nc.tensor.matmul(
                pescratch[0:1, 0:1], lhsT=a, rhs=b, start=True, stop=True
            )
            return bi.ins

        # ---- input1 load (SP ring) ----
        x1stage = stage_pool.tile([128, B * I // 128], f32)  # (128, 8192)
        x1_r = x1.rearrange("(n p q) i -> p n q i", p=128, q=2)  # (128,64,2,64)
        half = B * I // 128 // 2
        nhalf = NSUP // 2
        ring_dma(
            "sp",
            x1stage[:, 0:half].rearrange("p (n q i) -> p n q i", q=2, i=I),
            x1_r[:, 0:nhalf],
        )
        ring_dma(
            "sp",
            x1stage[:, half:].rearrange("p (n q i) -> p n q i", q=2, i=I),
            x1_r[:, nhalf:],
        )

        # ---- weights load (ACT ring) + K-reduction -> w2 (64, 256) ----
        w_flat = w.rearrange("i j k -> i (j k)")  # (64, 16384)
        w2 = const_pool.tile([64, JS], f32)
        csz = JS * K // NWCHUNK  # elems per chunk per partition
        jcs = JS // NWCHUNK      # w2 columns per chunk
        wchunks = []
        for c in range(NWCHUNK):
            if c >= WBUFS:
                # pre-carry the old chunk DMA's lane sem on the ring
                flag_dma("act", wchunks[c - WBUFS][0:1, 0:1])
            wchunk = wc_pool.tile([64, csz], f32, name=f"wchunk{c}", tag="wchunk")
            wchunks.append(wchunk)
            ring_dma("act", wchunk[:], w_flat[:, c * csz : (c + 1) * csz])
            nc.vector.tensor_reduce(
                out=w2[:, c * jcs : (c + 1) * jcs],
                in_=wchunk[:].rearrange("p (j k) -> p j k", k=K),
                axis=mybir.AxisListType.X,
                op=mybir.AluOpType.add,
            )

        # joiner: PE observes identity's production (gpsimd sem) once
        prev_join = pe_join(identity[0:1, 0:1])

        # ---- transposes: x1Tq[i, n*128+p] = x1[256n+2p+q, i] ----
        x1T = [
            const_pool.tile([64, NSUP * 128], f32, name=f"x1T{q}")
            for q in range(2)
        ]
        NB = NSUP // 4  # 16 transpose batches
        for m in range(NB):
            if m == NB // 2:
                j = pe_join(x1stage[0:1, half : half + 1])
                add_dep_helper(j, prev_join, False, "order joins")
                prev_join = j
            if m >= 1:
                # absorb the ACT-copy release of the psum slots reused now
                j = pe_join(
                    x1T[0][0:1, (m - 1) * 512 : (m - 1) * 512 + 1],
                    x1T[1][0:1, (m - 1) * 512 : (m - 1) * 512 + 1],
                )
                add_dep_helper(j, prev_join, False, "order joins")
                prev_join = j
            for q in range(2):
                tt = tr_pool.tile([64, 512], f32, name=f"tt{q}", tag=f"tt{q}")
                for s in range(4):
                    n = m * 4 + s
                    bi = nc.tensor.transpose(
                        tt[:, s * 128 : (s + 1) * 128],
                        x1stage[:, n * 128 + q * 64 : n * 128 + (q + 1) * 64],
                        identity[:],
                    )
                    if s == 0:
                        add_dep_helper(bi.ins, prev_join, False, "after join")
                nc.scalar.copy(x1T[q][:, m * 512 : (m + 1) * 512], tt[:])

        # joiner: single ACT-sem wait covering the last x1T copies
        jlast = pe_join(
            x1T[0][0:1, NSUP * 128 - 1 :],
            x1T[1][0:1, NSUP * 128 - 1 :],
        )
        add_dep_helper(jlast, prev_join, False, "order joins")
        prev_join = jlast

        # ---- main loop ----
        x2_r = x2.rearrange(
            "(g s p q) j -> g p s q j", g=NG, s=GROUP, p=128, q=2
        )
        out_r = out.rearrange(
            "(g s p q) j -> g p s q j", g=NG, s=GROUP, p=128, q=2
        )

        xtiles = []
        otiles = []

        def load(g):
            assert len(xtiles) == g
            if g >= XBUFS:
                # pre-carry the old xtile load's lane sem on the ring
                flag_dma("sp", xtiles[g - XBUFS][0:1, 0:1])
            xt = x_pool.tile([128, GROUP * 2 * JS], f32, name=f"xt{g}", tag="xt")
            xtiles.append(xt)
            ring_dma(
                "sp",
                xt[:].rearrange("p (s q j) -> p s q j", s=GROUP, q=2),
                x2_r[g],
            )

        for g in range(min(XBUFS, NG)):
            load(g)

        for g in range(NG):
            xtile = xtiles[g]
            ot = o_pool.tile([128, GROUP * 2 * JS], f32, name=f"ot{g}", tag="ot")
            otiles.append(ot)
            # DVE observes this load's DMA semaphore once
            jv1 = nc.vector.tensor_copy(
                dvescratch[0:1, 2 * g : 2 * g + 1], xtile[0:1, 0:1]
            ).ins
            jv2 = None
            if g >= OBUFS:
                # DVE observes the old otile store's lane sem once
                jv2 = nc.vector.tensor_copy(
                    otiles[g - OBUFS][0:1, 0:1],
                    dvescratch[0:1, 2 * g : 2 * g + 1],
                ).ins
            for s in range(GROUP):
                n = g * GROUP + s
                if n >= 4:
                    # absorb the DVE release of the psum slot reused now
                    g4, s4 = divmod(n - 4, GROUP)
                    j = pe_join(
                        otiles[g4][0:1, s4 * 512 : s4 * 512 + 1]
                    )
                    add_dep_helper(j, prev_join, False, "order joins")
                    prev_join = j
                pt = ps_pool.tile([128, 2 * JS], f32)  # (128, 512) = 1 bank
                for q in range(2):
                    bi = nc.tensor.matmul(
                        pt[:, q * JS : (q + 1) * JS],
                        lhsT=x1T[q][:, n * 128 : (n + 1) * 128],
                        rhs=w2[:],
                        start=True,
                        stop=True,
                    )
                    if q == 0:
                        add_dep_helper(bi.ins, prev_join, False, "after join")
                bi = nc.vector.tensor_mul(
                    ot[:, s * 512 : (s + 1) * 512],
                    pt[:],
                    xtile[:, s * 512 : (s + 1) * 512],
                )
                if s == 0:
                    add_dep_helper(bi.ins, jv1, False, "after join")
                    if jv2 is not None:
                        add_dep_helper(bi.ins, jv2, False, "after join")
            ring_dma(
                "sp",
                out_r[g],
                ot[:].rearrange("p (s q j) -> p s q j", s=GROUP, q=2),
            )
            if g + XBUFS < NG:
                load(g + XBUFS)

    nc.compile()
    return nc


def _get_nc():
    if "nc" not in _CACHE:
        _CACHE["nc"] = _build_nc()
    return _CACHE["nc"]


def _make_in_maps(input1, input2, weights):
    input1 = np.ascontiguousarray(input1, dtype=np.float32)
    in_maps = []
    for c in range(NCORES):
        sl = slice(c * JS, (c + 1) * JS)
        in_maps.append(
            {
                "input1": input1,
                "input2": np.ascontiguousarray(input2[:, sl], dtype=np.float32),
                "weights": np.ascontiguousarray(weights[:, sl, :], dtype=np.float32),
            }
        )
    return in_maps


def run(input1, input2, weights, trace=False, **spmd_kwargs):
    from concourse.bass_utils import run_bass_kernel_spmd

    nc = _get_nc()
    in_maps = _make_in_maps(input1, input2, weights)
    res = run_bass_kernel_spmd(
        nc, in_maps, core_ids=list(range(NCORES)), trace=trace, **spmd_kwargs
    )
    outs = [res.results[c]["out"] for c in range(NCORES)]
    full = np.concatenate(outs, axis=1)
    return full, res


def kernel(input1, input2, weights):
    full, _ = run(input1, input2, weights, trace=False)
    return full


# revision 21
# speedup vs baseline: 1.4043x; 1.4043x over previous
"""Trainium2 Bass kernel for nn_Bilinear_54065048322517.

Math:  out[b, j] = input2[b, j] * sum_{i,k} weights[i, j, k] * input1[b, i]
           =   input2 * (input1 @ weights.sum(axis=2))
Shapes: input1 (16384, 64) f32, input2 (16384, 2048) f32,
        weights (64, 2048, 64) f32, out (16384, 2048) f32.

Sharding: split J=2048 into 8 shards of 256 (one per NeuronCore).
Each core reads: input1 full (4MB) + its input2 shard (16MB) + its
weights shard (4MB), writes its out shard (16MB) -> 40MB of HBM
traffic per core (vs 64.5MB for batch sharding, which would have to
replicate the 32MB weights).

Built on bacc.Bacc + TileContext; Bacc.compile() legalizes the
one-embedded-wait-per-instruction TRN2 constraint by splitting extra
waits into event-semaphore instructions.

Per-core kernel:
  phase A:
    - weights shard (64,256,64) loaded as 8 chunks (64, 2048) on the
      ACT ring; DVE grouped-reduce over K -> w2 (64, 256)
    - input1 loaded as (128, 8192) on the SP ring: partition p holds
      rows {256n + 2p + q} (512B contiguous runs); 128 TensorE
      transposes of (128, 64) blocks -> x1T0/x1T1 (64, 8192):
      x1Tq[i, n*128 + p] = x1[256n + 2p + q, i].
  phase B (groups of 4 super-tiles = 1024 rows):
    - DMA x2 group -> xtile (128, 2048), SP ring (2KB runs)
    - per super-tile n: 2 matmuls in float32r (K=64, M=128, N=256):
        pt[:, q*256:(q+1)*256] = x1Tq[:, n*128:+128].T @ w2
      (psum partition p of q-half <-> row 256n + 2p + q)
    - DVE: otile = pt * xtile
    - DMA otile -> out group, SP ring.
"""

import numpy as np

B, I, J, K = 16384, 64, 2048, 64
NCORES = 8
JS = J // NCORES          # 256 columns per core
NSUP = B // 256           # 64 super-tiles of 256 rows
GROUP = 4                 # super-tiles per DMA group (1MB per stream DMA)
NG = NSUP // GROUP        # 16 groups
NWCHUNK = 8               # weights load chunks
WBUFS = 4                 # weight chunk slots
XBUFS = 3                 # xtile buffer depth
OBUFS = 2                 # otile buffer depth

_CACHE = {}


def _build_nc():
    from contextlib import ExitStack

    import concourse.mybir as mybir
    import concourse.tile as tile
    from concourse import bacc, masks

    f32 = mybir.dt.float32
    f32r = mybir.dt.float32r
    nc = bacc.Bacc()

    x1 = nc.dram_tensor("input1", [B, I], f32, kind="ExternalInput")
    x2 = nc.dram_tensor("input2", [B, JS], f32, kind="ExternalInput")
    w = nc.dram_tensor("weights", [I, JS, K], f32, kind="ExternalInput")
    out = nc.dram_tensor("out", [B, JS], f32, kind="ExternalOutput")

    with tile.TileContext(nc) as tc, ExitStack() as ctx:
        const_pool = ctx.enter_context(tc.tile_pool(name="const", bufs=1))
        stage_pool = ctx.enter_context(tc.tile_pool(name="stage", bufs=1))
        wc_pool = ctx.enter_context(tc.tile_pool(name="wc", bufs=WBUFS))
        x_pool = ctx.enter_context(tc.tile_pool(name="xin", bufs=XBUFS))
        o_pool = ctx.enter_context(tc.tile_pool(name="oout", bufs=OBUFS))
        ps_pool = ctx.enter_context(tc.tile_pool(name="ps", bufs=4, space="PSUM"))
        tr_pool = ctx.enter_context(tc.tile_pool(name="tr", bufs=2, space="PSUM"))

        identity = const_pool.tile([128, 128], f32)
        masks.make_identity(nc, identity[:])

        # ---- input1 load (SP ring) ----
        x1stage = stage_pool.tile([128, B * I // 128], f32)  # (128, 8192)
        x1_r = x1.rearrange("(n p q) i -> p n q i", p=128, q=2)  # (128,64,2,64)
        half = B * I // 128 // 2
        nhalf = NSUP // 2
        nc.sync.dma_start(
            out=x1stage[:, 0:half].rearrange("p (n q i) -> p n q i", q=2, i=I),
            in_=x1_r[:, 0:nhalf],
        )
        nc.sync.dma_start(
            out=x1stage[:, half:].rearrange("p (n q i) -> p n q i", q=2, i=I),
            in_=x1_r[:, nhalf:],
        )

        # ---- weights load (ACT ring) + K-reduction -> w2 (64, 256) ----
        w_flat = w.rearrange("i j k -> i (j k)")  # (64, 16384)
        # float32r so the fp32r matmuls get pre-rounded operands; the DVE
        # accumulates in fp32 internally and only rounds the final write.
        w2 = const_pool.tile([64, JS], f32r)
        csz = JS * K // NWCHUNK  # elems per chunk per partition
        jcs = JS // NWCHUNK      # w2 columns per chunk
        for c in range(NWCHUNK):
            wchunk = wc_pool.tile([64, csz], f32, name=f"wchunk{c}", tag="wchunk")
            nc.scalar.dma_start(
                out=wchunk[:], in_=w_flat[:, c * csz : (c + 1) * csz]
            )
            with nc.allow_low_precision(
                "fp32r rounding on final write only; DVE accumulates fp32"
            ):
                nc.vector.tensor_reduce(
                    out=w2[:, c * jcs : (c + 1) * jcs],
                    in_=wchunk[:].rearrange("p (j k) -> p j k", k=K),
                    axis=mybir.AxisListType.X,
                    op=mybir.AluOpType.add,
                )

        # ---- transposes: x1Tq[i, n*128+p] = x1[256n+2p+q, i] ----
        x1T = [
            const_pool.tile([64, NSUP * 128], f32r, name=f"x1T{q}")
            for q in range(2)
        ]
        NB = NSUP // 4  # 16 transpose batches
        for m in range(NB):
            for q in range(2):
                tt = tr_pool.tile([64, 512], f32, name=f"tt{q}", tag=f"tt{q}")
                for s in range(4):
                    n = m * 4 + s
                    nc.tensor.transpose(
                        tt[:, s * 128 : (s + 1) * 128],
                        x1stage[:, n * 128 + q * 64 : n * 128 + (q + 1) * 64],
                        identity[:],
                    )
                nc.scalar.copy(x1T[q][:, m * 512 : (m + 1) * 512], tt[:])

        # ---- main loop ----
        x2_r = x2.rearrange(
            "(g s p q) j -> g p s q j", g=NG, s=GROUP, p=128, q=2
        )
        out_r = out.rearrange(
            "(g s p q) j -> g p s q j", g=NG, s=GROUP, p=128, q=2
        )

        xtiles = []

        def load(g):
            assert len(xtiles) == g
            xt = x_pool.tile([128, GROUP * 2 * JS], f32, name=f"xt{g}", tag="xt")
            xtiles.append(xt)
            nc.sync.dma_start(
                out=xt[:].rearrange("p (s q j) -> p s q j", s=GROUP, q=2),
                in_=x2_r[g],
            )

        for g in range(min(XBUFS, NG)):
            load(g)

        for g in range(NG):
            xtile = xtiles[g]
            ot = o_pool.tile([128, GROUP * 2 * JS], f32, name=f"ot{g}", tag="ot")
            for s in range(GROUP):
                n = g * GROUP + s
                pt = ps_pool.tile([128, 2 * JS], f32)  # (128, 512) = 1 bank
                for q in range(2):
                    nc.tensor.matmul(
                        pt[:, q * JS : (q + 1) * JS],
                        lhsT=x1T[q][:, n * 128 : (n + 1) * 128],
                        rhs=w2[:],
                        start=True,
                        stop=True,
                    )
                nc.vector.tensor_mul(
                    ot[:, s * 512 : (s + 1) * 512],
                    pt[:],
                    xtile[:, s * 512 : (s + 1) * 512],
                )
            nc.sync.dma_start(
                out=out_r[g],
                in_=ot[:].rearrange("p (s q j) -> p s q j", s=GROUP, q=2),
            )
            if g + XBUFS < NG:
                load(g + XBUFS)

    nc.compile()
    return nc


def _get_nc():
    if "nc" not in _CACHE:
        _CACHE["nc"] = _build_nc()
    return _CACHE["nc"]


def _make_in_maps(input1, input2, weights):
    input1 = np.ascontiguousarray(input1, dtype=np.float32)
    in_maps = []
    for c in range(NCORES):
        sl = slice(c * JS, (c + 1) * JS)
        in_maps.append(
            {
                "input1": input1,
                "input2": np.ascontiguousarray(input2[:, sl], dtype=np.float32),
                "weights": np.ascontiguousarray(weights[:, sl, :], dtype=np.float32),
            }
        )
    return in_maps


def run(input1, input2, weights, trace=False, **spmd_kwargs):
    from concourse.bass_utils import run_bass_kernel_spmd

    nc = _get_nc()
    in_maps = _make_in_maps(input1, input2, weights)
    res = run_bass_kernel_spmd(
        nc, in_maps, core_ids=list(range(NCORES)), trace=trace, **spmd_kwargs
    )
    outs = [res.results[c]["out"] for c in range(NCORES)]
    full = np.concatenate(outs, axis=1)
    return full, res


def kernel(input1, input2, weights):
    full, _ = run(input1, input2, weights, trace=False)
    return full
